# revision 16
# baseline (speedup 1.0000x reference)
"""Trainium2 Bass kernel for ConditionalFeedForward (MoE routed FFN).

Problem: M=2048 tokens, D=1024, I=2048, E=8 experts, TOPK=2.
out[t, s] = FFN_{e}(x[t]) with e = expert_indices[t, s], where
FFN_e(x) = (silu(x @ w1_e.T) * (x @ w3_e.T)) @ w2_e.T  (w13 = [w1; w3]).

Strategy (expert parallelism, 8 experts -> 8 cores):
 - Host routes (token, slot) pairs to the core owning the selected expert,
   pads each core's token batch to a common capacity C, and transposes
   activations so features live on SBUF partitions and tokens on the free
   dim.  No device collectives needed: the "all-to-all" is a host gather
   and scatter around one SPMD kernel launch.
 - Everything on the PE path is bf16 (PE streams 1 col/cycle for both bf16
   and fp32r, but bf16 halves HBM traffic: 12 MB of weights per core fits
   under the PE time with slack, so the whole weight set is prefetched
   into SBUF and the PE never stalls mid-stream).  PSUM accumulation is
   fp32; silu*gate runs on ACT+DVE in fp32 and requantizes g to bf16.
 - Startup critical path: the first matmul needs only x[k=0] and the first
   half of pair-0's w1 block, so those land as small DMAs issued first,
   split across the two HWDGE queues (sync + scalar).
 - Tail: the last output d-block is processed in two column chunks so the
   final PSUM->SBUF copy + DMA-out overlaps the preceding matmuls.
"""

import os

import numpy as np
import ml_dtypes

import concourse.bass as bass
import concourse.tile as tile
from concourse import bacc, mybir
from concourse.bass_utils import run_bass_kernel_spmd

M, D, I, E, TOPK = 2048, 1024, 2048, 8, 2
P = 128
KD = D // P            # 8   k-tiles over D (mm1 contraction)
NI2 = (2 * I) // P     # 32  n-tiles over 2I (mm1 output rows)
NPAIR = NI2 // 2       # 16  (x1, x3) pairs
KI = I // P            # 16  k-tiles over I (mm2 contraction)
ND = D // P            # 8   d-tiles over D (mm2 output rows)

F32 = mybir.dt.float32
BF16 = mybir.dt.bfloat16
NP_BF16 = ml_dtypes.bfloat16

# exec time of the most recent launch (ns), populated when BASS_TRACE=1
LAST_EXEC_TIME_NS = None

_program_cache = {}


def _chunks_for(C):
    """Split C token-columns into matmul moving-dim chunks (<=512 each)."""
    n_ch = -(-C // 512)
    base = -(-C // (n_ch * 32)) * 32
    chunks = []
    off = 0
    while off < C:
        cn = min(base, C - off)
        chunks.append((off, cn))
        off += cn
    return tuple(chunks)


def _build_program(C):
    chunks = _chunks_for(C)
    nc = bacc.Bacc(
        "TRN2",
        target_bir_lowering=False,
        debug=False,
        enable_asserts=False,
        num_devices=E,
    )

    # xw0: startup-critical payload packed into one partition-major tensor
    # so it moves as TWO large-element DMAs (big contiguous runs per
    # partition are what the DMA engines stream fastest, especially during
    # the first ~6us while the DMA path is still ramping up):
    #   [ x k0-3 | wA0 k0-3 | wB0 k0-3 | x k4-7 | wA0 k4-7 | wB0 k4-7 ]
    # w13: (x1, x3) row-block PAIRS fused per partition row (pairs 1..15)
    # w2: two d-blocks fused per row
    XW = KD * C + 2 * KD * P
    H1 = 4 * C + KD * P          # end of piece 1
    xw0_d = nc.dram_tensor("xw0", (P, XW), BF16, kind="ExternalInput").ap()
    w13_d = nc.dram_tensor(
        "w13t", (NPAIR - 1, P, 2 * KD * P), BF16, kind="ExternalInput"
    ).ap()
    w2_d = nc.dram_tensor(
        "w2t", (ND // 2, P, 2 * KI * P), BF16, kind="ExternalInput"
    ).ap()
    out_d = nc.dram_tensor("outT", (ND, P, C), F32, kind="ExternalOutput").ap()

    with tile.TileContext(nc) as tc:
        with (
            tc.tile_pool(name="xg", bufs=1) as xg_pool,
            tc.tile_pool(name="wt", bufs=1) as w_pool,
            tc.tile_pool(name="tmp", bufs=4) as tmp_pool,
            tc.tile_pool(name="ps", bufs=8, space="PSUM") as ps_pool,
        ):
            # ---- PE warmup: dummy matmuls on (uninitialized) SBUF ----
            # The HAM clock gate holds the PE at 1.2 GHz until it has been
            # busy ~3.4us.  Real data can't arrive before ~10us (queue
            # prologue + DMA first-byte latency + DMA-path ramp), so burn
            # that window on scratch matmuls: the PE hits 2.4 GHz right as
            # real matmuls start.  Results land in scratch PSUM, never read.
            # Warmup needs >=3.4us of SUSTAINED matmul activity to fire the
            # HAM clock-gate (10 x ~0.43us cold matmuls); after that the PE
            # may sit idle up to ~3.4us (one MID window) without being
            # re-throttled, which comfortably covers the gap until the
            # first x/w piece lands (~12.5-14us).
            zt = xg_pool.tile([P, 256], BF16, tag="zt", name="zt")
            nc.vector.memset(zt[:], 0.0)
            for _ in range(16):
                psw = ps_pool.tile([P, 256], F32, tag="ps", name="ps")
                nc.tensor.matmul(
                    psw, zt[:, :P], zt[:], start=True, stop=True
                )

            # ---- all DMAs issued upfront, most-urgent first ----
            # The sync-queue HWDGE ring measures ~420 GB/s once streaming;
            # the scalar-queue ring only ~70 GB/s, so EVERYTHING goes on
            # sync, in exactly need-order.
            xw0 = xg_pool.tile([P, XW], BF16, tag="x", name="x")

            def x_tile(k):
                off = k * C + (KD * P if k >= 4 else 0)
                return xw0[:, off : off + C]

            def w0_tile(half, k):
                # half 0 -> wA0 slice, half 1 -> wB0 slice, for k-tile k
                off = (8 * C + KD * P if k >= 4 else 4 * C) + half * (
                    KD * P // 2
                ) + (k % 4) * P
                return xw0[:, off : off + P]

            x_tiles = [x_tile(k) for k in range(KD)]
            w13s = {
                pr: w_pool.tile(
                    [P, 2 * KD * P], BF16, tag=f"w13_{pr}", name=f"w13_{pr}"
                )
                for pr in range(1, NPAIR)
            }
            w2s = [
                w_pool.tile(
                    [P, 2 * KI * P], BF16, tag=f"w2_{j}", name=f"w2_{j}"
                )
                for j in range(ND // 2)
            ]

            nc.sync.dma_start(xw0[:, :H1], xw0_d[:, :H1])
            nc.sync.dma_start(xw0[:, H1:], xw0_d[:, H1:])
            for pr in range(1, NPAIR):
                nc.sync.dma_start(w13s[pr][:], w13_d[pr - 1])
            for j in range(ND // 2):
                nc.sync.dma_start(w2s[j][:], w2_d[j])

            g_tiles = [
                xg_pool.tile([P, C], BF16, tag=f"g{ki}", name=f"g{ki}")
                for ki in range(KI)
            ]

            # ---- mm1 + silu*gate: process (x1, x3) row-block pairs ----
            # pair 0 interleaves the psA/psB k-loops so each arriving x
            # k-tile feeds two matmuls back-to-back (PE keeps pace with the
            # x stream instead of stalling then bursting).
            for pr in range(NPAIR):
                if pr == 0:
                    lhsT = w0_tile
                else:
                    slab = w13s[pr]
                    lhsT = lambda half, k, slab=slab: slab[
                        :, half * KD * P + k * P : half * KD * P + (k + 1) * P
                    ]
                for c0, cn in chunks:
                    psA = ps_pool.tile([P, 512], F32, tag="ps", name="ps")[:, :cn]
                    psB = ps_pool.tile([P, 512], F32, tag="ps", name="ps")[:, :cn]
                    if pr == 0:
                        for k in range(KD):
                            for half, ps_ in ((0, psA), (1, psB)):
                                nc.tensor.matmul(
                                    ps_,
                                    lhsT(half, k),
                                    x_tiles[k][:, c0 : c0 + cn],
                                    start=(k == 0),
                                    stop=(k == KD - 1),
                                )
                    else:
                        for half, ps_ in ((0, psA), (1, psB)):
                            for k in range(KD):
                                nc.tensor.matmul(
                                    ps_,
                                    lhsT(half, k),
                                    x_tiles[k][:, c0 : c0 + cn],
                                    start=(k == 0),
                                    stop=(k == KD - 1),
                                )
                    s = tmp_pool.tile([P, 512], F32, tag="s", name="s")[:, :cn]
                    nc.scalar.activation(s, psA, mybir.ActivationFunctionType.Silu)
                    nc.vector.tensor_mul(
                        out=g_tiles[pr][:, c0 : c0 + cn],
                        in0=s,
                        in1=psB,
                    )

            # ---- mm2: outT[d-block] = sum_ki w2T-tile @ g ----
            # last d-block runs in two half-chunks so its copy+DMA-out
            # overlaps compute instead of sitting in the kernel tail.
            for d in range(ND):
                wDD = w2s[d // 2]
                wD = wDD[:, (d % 2) * KI * P : (d % 2 + 1) * KI * P]
                if d == ND - 1 and len(chunks) == 1 and C >= 256:
                    out_chunks = [(0, C - 64), (C - 64, 64)]
                else:
                    out_chunks = chunks
                for c0, cn in out_chunks:
                    psO = ps_pool.tile([P, 512], F32, tag="ps", name="ps")[:, :cn]
                    for ki in range(KI):
                        nc.tensor.matmul(
                            psO,
                            wD[:, ki * P : (ki + 1) * P],
                            g_tiles[ki][:, c0 : c0 + cn],
                            start=(ki == 0),
                            stop=(ki == KI - 1),
                        )
                    ot = tmp_pool.tile([P, 512], F32, tag="o", name="o")[:, :cn]
                    nc.vector.tensor_copy(ot, psO)
                    nc.sync.dma_start(out_d[d][:, c0 : c0 + cn], ot)

    nc.compile()
    return nc


def _get_program(C):
    if C not in _program_cache:
        _program_cache[C] = _build_program(C)
    return _program_cache[C]


def _ensure_ntff_hook():
    """Provide antenv.axon_hooks if the image lacks it, so trace=True works."""
    import sys
    import types

    try:
        import antenv.axon_hooks  # noqa: F401

        return
    except ImportError:
        pass
    try:
        import antenv
        from trn_agent_boot.trn_boot import _ntff_profile_via_ctypes

        mod = types.ModuleType("antenv.axon_hooks")
        state = {"hook": None}
        mod.set_axon_ntff_profile_hook = lambda h: state.__setitem__("hook", h)
        mod.get_axon_ntff_profile_hook = lambda: state["hook"]
        sys.modules["antenv.axon_hooks"] = mod
        antenv.axon_hooks = mod
        mod.set_axon_ntff_profile_hook(
            _ntff_profile_via_ctypes("/opt/axon/libaxon_pjrt.so")
        )
    except Exception:
        pass


def kernel(x, w13, w2, expert_indices):
    global LAST_EXEC_TIME_NS
    x = np.asarray(x, dtype=np.float32)
    w13 = np.asarray(w13, dtype=np.float32)
    w2 = np.asarray(w2, dtype=np.float32)
    idx = np.asarray(expert_indices)
    idx32 = idx.astype(np.int64)

    m, d_model = x.shape
    e, two_i, _ = w13.shape
    inter = w2.shape[2]
    topk = idx.shape[1]
    assert (m, d_model, e, two_i, inter, topk) == (M, D, E, 2 * I, I, TOPK)

    # ---- host routing: unique (token, expert) work items per expert ----
    # A token picking the same expert in both slots computes the FFN once;
    # the result is scattered to every matching slot.
    tok_unique = [
        np.unique(np.concatenate([np.nonzero(idx32[:, s] == ei)[0] for s in range(topk)]))
        for ei in range(E)
    ]
    max_cnt = max(len(u) for u in tok_unique)
    C = max(256, int(max_cnt))

    nc = _get_program(C)

    in_maps = []
    for ei in range(E):
        tok_ids = tok_unique[ei]
        cnt = len(tok_ids)

        xg = np.zeros((C, D), dtype=np.float32)
        xg[:cnt] = x[tok_ids]
        xT = np.ascontiguousarray(
            xg.T.reshape(KD, P, C).transpose(1, 0, 2).astype(NP_BF16)
        )                                            # [p, k, c]

        A4 = w13[ei].astype(NP_BF16).reshape(NI2, P, KD, P)   # [n, c, k, p]
        w13t = A4.transpose(0, 3, 2, 1).reshape(NI2, P, KD * P)
        w13p = np.ascontiguousarray(
            np.concatenate([w13t[:NPAIR], w13t[NPAIR:]], axis=2)
        )                                            # [pair, p, 2*KD*P]

        # pair 0 rides with x in the packed startup tensor:
        # [ x k0-3 | wA0 k0-3 | wB0 k0-3 | x k4-7 | wA0 k4-7 | wB0 k4-7 ]
        H = KD * P // 2
        xw0 = np.concatenate(
            [
                xT[:, :4].reshape(P, 4 * C),
                w13p[0][:, :H],                      # wA0 k0-3
                w13p[0][:, KD * P : KD * P + H],     # wB0 k0-3
                xT[:, 4:].reshape(P, 4 * C),
                w13p[0][:, H : KD * P],              # wA0 k4-7
                w13p[0][:, KD * P + H :],            # wB0 k4-7
            ],
            axis=1,
        )

        B4 = w2[ei].astype(NP_BF16).reshape(ND, P, KI, P)     # [d, c, ki, p]
        w2t = B4.transpose(0, 3, 2, 1).reshape(ND, P, KI * P)
        w2p = np.ascontiguousarray(
            w2t.reshape(ND // 2, 2, P, KI * P).transpose(0, 2, 1, 3).reshape(
                ND // 2, P, 2 * KI * P
            )
        )                                            # [dpair, p, 2*KI*P]

        in_maps.append({"xw0": xw0, "w13t": w13p[1:], "w2t": w2p})

    trace = bool(os.environ.get("BASS_TRACE"))
    if trace:
        _ensure_ntff_hook()
    res = run_bass_kernel_spmd(nc, in_maps, core_ids=list(range(E)), trace=trace)
    LAST_EXEC_TIME_NS = res.exec_time_ns

    # ---- host scatter: copy each expert's outputs to all matching slots ----
    out = np.empty((M, topk, D), dtype=np.float32)
    for ei in range(E):
        outT = res.results[ei]["outT"].reshape(D, C)
        oe = outT[:, : len(tok_unique[ei])].T        # [cnt, D]
        for s in range(topk):
            sel = np.nonzero(idx32[:, s] == ei)[0]
            out[sel, s] = oe[np.searchsorted(tok_unique[ei], sel)]

    return out


# revision 17
# speedup vs baseline: 1.0309x; 1.0309x over previous
"""Trainium2 Bass kernel for ConditionalFeedForward (MoE routed FFN).

Problem: M=2048 tokens, D=1024, I=2048, E=8 experts, TOPK=2.
out[t, s] = FFN_{e}(x[t]) with e = expert_indices[t, s], where
FFN_e(x) = (silu(x @ w1_e.T) * (x @ w3_e.T)) @ w2_e.T  (w13 = [w1; w3]).

Strategy (expert parallelism, 8 experts -> 8 cores):
 - Host routes (token, slot) pairs to the core owning the selected expert,
   pads each core's token batch to a common capacity C, and transposes
   activations so features live on SBUF partitions and tokens on the free
   dim.  No device collectives needed: the "all-to-all" is a host gather
   and scatter around one SPMD kernel launch.
 - Everything on the PE path is bf16 (PE streams 1 col/cycle for both bf16
   and fp32r, but bf16 halves HBM traffic: 12 MB of weights per core fits
   under the PE time with slack, so the whole weight set is prefetched
   into SBUF and the PE never stalls mid-stream).  PSUM accumulation is
   fp32; silu*gate runs on ACT+DVE in fp32 and requantizes g to bf16.
 - Startup critical path: the first matmul needs only x[k=0] and the first
   half of pair-0's w1 block, so those land as small DMAs issued first,
   split across the two HWDGE queues (sync + scalar).
 - Tail: the last output d-block is processed in two column chunks so the
   final PSUM->SBUF copy + DMA-out overlaps the preceding matmuls.
"""

import os

import numpy as np
import ml_dtypes

import concourse.bass as bass
import concourse.tile as tile
from concourse import bacc, mybir
from concourse.bass_utils import run_bass_kernel_spmd

M, D, I, E, TOPK = 2048, 1024, 2048, 8, 2
P = 128
KD = D // P            # 8   k-tiles over D (mm1 contraction)
NI2 = (2 * I) // P     # 32  n-tiles over 2I (mm1 output rows)
NPAIR = NI2 // 2       # 16  (x1, x3) pairs
KI = I // P            # 16  k-tiles over I (mm2 contraction)
ND = D // P            # 8   d-tiles over D (mm2 output rows)

F32 = mybir.dt.float32
BF16 = mybir.dt.bfloat16
NP_BF16 = ml_dtypes.bfloat16

# exec time of the most recent launch (ns), populated when BASS_TRACE=1
LAST_EXEC_TIME_NS = None

_program_cache = {}


def _chunks_for(C):
    """Split C token-columns into matmul moving-dim chunks (<=512 each)."""
    n_ch = -(-C // 512)
    base = -(-C // (n_ch * 32)) * 32
    chunks = []
    off = 0
    while off < C:
        cn = min(base, C - off)
        chunks.append((off, cn))
        off += cn
    return tuple(chunks)


def _build_program(C):
    chunks = _chunks_for(C)
    nc = bacc.Bacc(
        "TRN2",
        target_bir_lowering=False,
        debug=False,
        enable_asserts=False,
        num_devices=E,
    )

    # xw0: startup-critical payload packed into one partition-major tensor
    # so it moves as TWO large-element DMAs (big contiguous runs per
    # partition are what the DMA engines stream fastest, especially during
    # the first ~6us while the DMA path is still ramping up):
    #   [ x k0-3 | wA0 k0-3 | wB0 k0-3 | x k4-7 | wA0 k4-7 | wB0 k4-7 ]
    # w13: (x1, x3) row-block PAIRS fused per partition row (pairs 1..15)
    # w2: two d-blocks fused per row
    XW = KD * C + 2 * KD * P
    H1 = 4 * C + KD * P          # end of piece 1
    xw0_d = nc.dram_tensor("xw0", (P, XW), BF16, kind="ExternalInput").ap()
    w13_d = nc.dram_tensor(
        "w13t", (NPAIR - 1, P, 2 * KD * P), BF16, kind="ExternalInput"
    ).ap()
    w2_d = nc.dram_tensor(
        "w2t", (ND // 2, P, 2 * KI * P), BF16, kind="ExternalInput"
    ).ap()
    out_d = nc.dram_tensor("outT", (ND, P, C), F32, kind="ExternalOutput").ap()

    with tile.TileContext(nc) as tc:
        with (
            tc.tile_pool(name="xg", bufs=1) as xg_pool,
            tc.tile_pool(name="wt", bufs=1) as w_pool,
            tc.tile_pool(name="tmp", bufs=4) as tmp_pool,
            tc.tile_pool(name="ps", bufs=8, space="PSUM") as ps_pool,
        ):
            # ---- PE warmup: dummy matmuls on (uninitialized) SBUF ----
            # The HAM clock gate holds the PE at 1.2 GHz until it has been
            # busy ~3.4us.  Real data can't arrive before ~10us (queue
            # prologue + DMA first-byte latency + DMA-path ramp), so burn
            # that window on scratch matmuls: the PE hits 2.4 GHz right as
            # real matmuls start.  Results land in scratch PSUM, never read.
            # Warmup needs >=3.4us of SUSTAINED matmul activity to fire the
            # HAM clock-gate (10 x ~0.43us cold matmuls); after that the PE
            # may sit idle up to ~3.4us (one MID window) without being
            # re-throttled, which comfortably covers the gap until the
            # first x/w piece lands (~12.5-14us).
            zt = xg_pool.tile([P, 256], BF16, tag="zt", name="zt")
            nc.vector.memset(zt[:], 0.0)
            for _ in range(20):
                psw = ps_pool.tile([P, 256], F32, tag="ps", name="ps")
                nc.tensor.matmul(
                    psw, zt[:, :P], zt[:], start=True, stop=True
                )

            # ---- all DMAs issued upfront, most-urgent first ----
            # The sync-queue HWDGE ring measures ~420 GB/s once streaming;
            # the scalar-queue ring only ~70 GB/s, so EVERYTHING goes on
            # sync, in exactly need-order.
            xw0 = xg_pool.tile([P, XW], BF16, tag="x", name="x")

            def x_tile(k):
                off = k * C + (KD * P if k >= 4 else 0)
                return xw0[:, off : off + C]

            def w0_tile(half, k):
                # half 0 -> wA0 slice, half 1 -> wB0 slice, for k-tile k
                off = (8 * C + KD * P if k >= 4 else 4 * C) + half * (
                    KD * P // 2
                ) + (k % 4) * P
                return xw0[:, off : off + P]

            x_tiles = [x_tile(k) for k in range(KD)]
            w13s = {
                pr: w_pool.tile(
                    [P, 2 * KD * P], BF16, tag=f"w13_{pr}", name=f"w13_{pr}"
                )
                for pr in range(1, NPAIR)
            }
            w2s = [
                w_pool.tile(
                    [P, 2 * KI * P], BF16, tag=f"w2_{j}", name=f"w2_{j}"
                )
                for j in range(ND // 2)
            ]

            nc.sync.dma_start(xw0[:, :H1], xw0_d[:, :H1])
            nc.sync.dma_start(xw0[:, H1:], xw0_d[:, H1:])
            for pr in range(1, NPAIR):
                nc.sync.dma_start(w13s[pr][:], w13_d[pr - 1])
            for j in range(ND // 2):
                nc.sync.dma_start(w2s[j][:], w2_d[j])

            g_tiles = [
                xg_pool.tile([P, C], BF16, tag=f"g{ki}", name=f"g{ki}")
                for ki in range(KI)
            ]

            # ---- mm1 + silu*gate: process (x1, x3) row-block pairs ----
            # pair 0 interleaves the psA/psB k-loops so each arriving x
            # k-tile feeds two matmuls back-to-back (PE keeps pace with the
            # x stream instead of stalling then bursting).
            for pr in range(NPAIR):
                if pr == 0:
                    lhsT = w0_tile
                else:
                    slab = w13s[pr]
                    lhsT = lambda half, k, slab=slab: slab[
                        :, half * KD * P + k * P : half * KD * P + (k + 1) * P
                    ]
                for c0, cn in chunks:
                    psA = ps_pool.tile([P, 512], F32, tag="ps", name="ps")[:, :cn]
                    psB = ps_pool.tile([P, 512], F32, tag="ps", name="ps")[:, :cn]
                    if pr == 0:
                        for k in range(KD):
                            for half, ps_ in ((0, psA), (1, psB)):
                                nc.tensor.matmul(
                                    ps_,
                                    lhsT(half, k),
                                    x_tiles[k][:, c0 : c0 + cn],
                                    start=(k == 0),
                                    stop=(k == KD - 1),
                                )
                    else:
                        for half, ps_ in ((0, psA), (1, psB)):
                            for k in range(KD):
                                nc.tensor.matmul(
                                    ps_,
                                    lhsT(half, k),
                                    x_tiles[k][:, c0 : c0 + cn],
                                    start=(k == 0),
                                    stop=(k == KD - 1),
                                )
                    s = tmp_pool.tile([P, 512], F32, tag="s", name="s")[:, :cn]
                    nc.scalar.activation(s, psA, mybir.ActivationFunctionType.Silu)
                    nc.vector.tensor_mul(
                        out=g_tiles[pr][:, c0 : c0 + cn],
                        in0=s,
                        in1=psB,
                    )

            # ---- mm2: outT[d-block] = sum_ki w2T-tile @ g ----
            # last d-block runs in two half-chunks so its copy+DMA-out
            # overlaps compute instead of sitting in the kernel tail.
            for d in range(ND):
                wDD = w2s[d // 2]
                wD = wDD[:, (d % 2) * KI * P : (d % 2 + 1) * KI * P]
                if d == ND - 1 and len(chunks) == 1 and C >= 256:
                    out_chunks = [(0, C - 64), (C - 64, 64)]
                else:
                    out_chunks = chunks
                for c0, cn in out_chunks:
                    psO = ps_pool.tile([P, 512], F32, tag="ps", name="ps")[:, :cn]
                    for ki in range(KI):
                        nc.tensor.matmul(
                            psO,
                            wD[:, ki * P : (ki + 1) * P],
                            g_tiles[ki][:, c0 : c0 + cn],
                            start=(ki == 0),
                            stop=(ki == KI - 1),
                        )
                    ot = tmp_pool.tile([P, 512], F32, tag="o", name="o")[:, :cn]
                    nc.vector.tensor_copy(ot, psO)
                    nc.sync.dma_start(out_d[d][:, c0 : c0 + cn], ot)

    nc.compile()
    return nc


def _get_program(C):
    if C not in _program_cache:
        _program_cache[C] = _build_program(C)
    return _program_cache[C]


def _ensure_ntff_hook():
    """Provide antenv.axon_hooks if the image lacks it, so trace=True works."""
    import sys
    import types

    try:
        import antenv.axon_hooks  # noqa: F401

        return
    except ImportError:
        pass
    try:
        import antenv
        from trn_agent_boot.trn_boot import _ntff_profile_via_ctypes

        mod = types.ModuleType("antenv.axon_hooks")
        state = {"hook": None}
        mod.set_axon_ntff_profile_hook = lambda h: state.__setitem__("hook", h)
        mod.get_axon_ntff_profile_hook = lambda: state["hook"]
        sys.modules["antenv.axon_hooks"] = mod
        antenv.axon_hooks = mod
        mod.set_axon_ntff_profile_hook(
            _ntff_profile_via_ctypes("/opt/axon/libaxon_pjrt.so")
        )
    except Exception:
        pass


def kernel(x, w13, w2, expert_indices):
    global LAST_EXEC_TIME_NS
    x = np.asarray(x, dtype=np.float32)
    w13 = np.asarray(w13, dtype=np.float32)
    w2 = np.asarray(w2, dtype=np.float32)
    idx = np.asarray(expert_indices)
    idx32 = idx.astype(np.int64)

    m, d_model = x.shape
    e, two_i, _ = w13.shape
    inter = w2.shape[2]
    topk = idx.shape[1]
    assert (m, d_model, e, two_i, inter, topk) == (M, D, E, 2 * I, I, TOPK)

    # ---- host routing: unique (token, expert) work items per expert ----
    # A token picking the same expert in both slots computes the FFN once;
    # the result is scattered to every matching slot.
    tok_unique = [
        np.unique(np.concatenate([np.nonzero(idx32[:, s] == ei)[0] for s in range(topk)]))
        for ei in range(E)
    ]
    max_cnt = max(len(u) for u in tok_unique)
    C = max(256, int(max_cnt))

    nc = _get_program(C)

    in_maps = []
    for ei in range(E):
        tok_ids = tok_unique[ei]
        cnt = len(tok_ids)

        xg = np.zeros((C, D), dtype=np.float32)
        xg[:cnt] = x[tok_ids]
        xT = np.ascontiguousarray(
            xg.T.reshape(KD, P, C).transpose(1, 0, 2).astype(NP_BF16)
        )                                            # [p, k, c]

        A4 = w13[ei].astype(NP_BF16).reshape(NI2, P, KD, P)   # [n, c, k, p]
        w13t = A4.transpose(0, 3, 2, 1).reshape(NI2, P, KD * P)
        w13p = np.ascontiguousarray(
            np.concatenate([w13t[:NPAIR], w13t[NPAIR:]], axis=2)
        )                                            # [pair, p, 2*KD*P]

        # pair 0 rides with x in the packed startup tensor:
        # [ x k0-3 | wA0 k0-3 | wB0 k0-3 | x k4-7 | wA0 k4-7 | wB0 k4-7 ]
        H = KD * P // 2
        xw0 = np.concatenate(
            [
                xT[:, :4].reshape(P, 4 * C),
                w13p[0][:, :H],                      # wA0 k0-3
                w13p[0][:, KD * P : KD * P + H],     # wB0 k0-3
                xT[:, 4:].reshape(P, 4 * C),
                w13p[0][:, H : KD * P],              # wA0 k4-7
                w13p[0][:, KD * P + H :],            # wB0 k4-7
            ],
            axis=1,
        )

        B4 = w2[ei].astype(NP_BF16).reshape(ND, P, KI, P)     # [d, c, ki, p]
        w2t = B4.transpose(0, 3, 2, 1).reshape(ND, P, KI * P)
        w2p = np.ascontiguousarray(
            w2t.reshape(ND // 2, 2, P, KI * P).transpose(0, 2, 1, 3).reshape(
                ND // 2, P, 2 * KI * P
            )
        )                                            # [dpair, p, 2*KI*P]

        in_maps.append({"xw0": xw0, "w13t": w13p[1:], "w2t": w2p})

    trace = bool(os.environ.get("BASS_TRACE"))
    if trace:
        _ensure_ntff_hook()
    res = run_bass_kernel_spmd(nc, in_maps, core_ids=list(range(E)), trace=trace)
    LAST_EXEC_TIME_NS = res.exec_time_ns

    # ---- host scatter: copy each expert's outputs to all matching slots ----
    out = np.empty((M, topk, D), dtype=np.float32)
    for ei in range(E):
        outT = res.results[ei]["outT"].reshape(D, C)
        oe = outT[:, : len(tok_unique[ei])].T        # [cnt, D]
        for s in range(topk):
            sel = np.nonzero(idx32[:, s] == ei)[0]
            out[sel, s] = oe[np.searchsorted(tok_unique[ei], sel)]

    return out
